# revision 30
# baseline (speedup 1.0000x reference)
"""AdaptiveSpikingAttention on 8 TRN2 NeuronCores (Bass/Tile).

Sharding: the 4096 (batch, seq) rows are split across 8 cores — core c owns
batch c//2, half c%2 (512 rows). Projections, gate MLPs and spike counting
are row-local; the two cores of a batch exchange k/v spike counts with a
pair AllGather before the attention.

Key transform: the 20-step LIF spike recurrence acc(x, T) is a monotone
step function of x whose <=20 jump points depend only on (alpha, beta, T).
The jump points are bisected on the host from the scalar parameters; on
device each element needs 20 compares against per-row thresholds instead
of a sequential 20-step recurrence.

Softmax: scores only ever exist transposed ([col, row]); the row bound
M_i = scale*(q_i . kmean) + C is folded into the score matmul as an extra
contraction row. row-max >= row-mean keeps the denominator well away from
underflow, and C centers the exp arguments in fp32 range.
"""

import sys
import numpy as np

sys.path.insert(0, "/opt/trn_rl_repo")

import concourse.bass as bass
import concourse.bacc as bacc
import concourse.tile as tile
import concourse.mybir as mybir
from concourse.bass_utils import run_bass_kernel_spmd
from concourse.masks import make_identity

f32 = mybir.dt.float32
bf16 = mybir.dt.bfloat16
P = 128
R = 512           # rows per core
E = 512
H, D = 8, 64
S = 1024
NK = 20           # thresholds per tensor
T_MAX = 20
CSHIFT = 114.0    # exp-range centering constant
SCALE = float(D) ** -0.5

_compiled = None


# ----------------------------------------------------------------- host math
def _build_thr_table(alpha, beta):
    """thr[T-1, k-1]: smallest f32 x with count(x, T) >= k (64.0 if never)."""
    alpha = np.float32(alpha)
    beta = np.float32(beta)

    def counts(xs, T):
        xs = xs.astype(np.float32)
        v = np.zeros_like(xs)
        i = np.zeros_like(xs)
        acc = np.zeros_like(xs)
        for t in range(T_MAX):
            a = np.float32(1.0) if t < T else np.float32(0.0)
            i = alpha * i + xs * a
            v = beta * v + i
            s = (v >= 1.0).astype(np.float32)
            v = v * (1.0 - s)
            acc = acc + s * a
        return acc

    thr = np.full((T_MAX, T_MAX), np.float32(64.0), np.float32)
    for T in range(1, T_MAX + 1):
        los = np.full(T, -3, np.float32)
        his = np.full(T, 6, np.float32)
        ks = np.arange(1, T + 1)
        for _ in range(60):
            mids = ((los.astype(np.float64) + his) / 2).astype(np.float32)
            ge = counts(mids, T) >= ks
            his = np.where(ge, mids, his)
            los = np.where(ge, los, mids)
        thr[T - 1, :T] = his
    return thr


# -------------------------------------------------------------- device build
def _build_program():
    nc = bacc.Bacc("TRN2", target_bir_lowering=False, debug=False,
                   enable_asserts=True, num_devices=8)
    A = mybir.AluOpType
    AF = mybir.ActivationFunctionType
    X = mybir.AxisListType.X

    def dram(name, shape, dt=f32, kind="ExternalInput"):
        return nc.dram_tensor(name, shape, dt, kind=kind)

    xT_d = dram("xT", [E, R])
    Wq_d = dram("Wq", [E, E])
    Wk_d = dram("Wk", [E, E])
    Wv_d = dram("Wv", [E, E])
    Wo_d = dram("Wo_s", [E, E], bf16)
    bo_d = dram("bo_row", [1, E])
    gw1_d = dram("gW1", [E, 128]); gb1_d = dram("gb1", [128, 1])
    gg_d = dram("gg", [128, 1]); gbe_d = dram("gbe", [128, 1])
    gw2_d = dram("gW2", [128, 64]); gb2_d = dram("gb2", [64, 1])
    gw3_d = dram("gW3", [64, 1]); gb3_d = dram("gb3", [1, 1])
    cw1_d = dram("cW1", [E, 64]); cb1_d = dram("cb1", [64, 1])
    cg_d = dram("cg", [64, 1]); cbe_d = dram("cbe", [64, 1])
    cw2_d = dram("cW2", [64, 32]); cb2_d = dram("cb2", [32, 1])
    cw3_d = dram("cW3", [32, 1]); cb3_d = dram("cb3", [1, 1])
    pos_d = dram("pos_row", [1, R])
    tbl_d = dram("tbl_all", [NK, 3 * NK])
    tau1_d = dram("tau1", [NK, 1])
    tau2_d = dram("tau2", [NK, 1])
    out_d = dram("out", [R, E], kind="ExternalOutput")

    with tile.TileContext(nc) as tc:
        with (
            tc.tile_pool(name="w", bufs=1) as wpool,
            tc.tile_pool(name="sb", bufs=2) as pool,
            tc.tile_pool(name="row", bufs=1) as rowp,
            tc.tile_pool(name="cnt", bufs=1) as cpool,
            tc.tile_pool(name="psA", bufs=2, space="PSUM") as psA,
            tc.tile_pool(name="psB", bufs=2, space="PSUM") as psB,
            tc.tile_pool(name="psC", bufs=1, space="PSUM") as psC,
            tc.tile_pool(name="dram", bufs=1, space="DRAM") as dpool,
        ):
            # ---------- load everything
            xT = wpool.tile([P, 4, R], f32)
            for c in range(4):
                nc.sync.dma_start(xT[:, c], xT_d[c * P:(c + 1) * P, :])
            Ws = {}
            for qi, (nm, d) in enumerate((("q", Wq_d), ("k", Wk_d),
                                          ("v", Wv_d))):
                W = wpool.tile([P, 4, E], f32, tag=f"W{nm}")
                eng = (nc.gpsimd, nc.scalar, nc.gpsimd)[qi]
                for c in range(4):
                    eng.dma_start(W[:, c], d[c * P:(c + 1) * P, :])
                Ws[nm] = W
            Wo = wpool.tile([D, H, E], bf16)
            for h in range(H):
                nc.scalar.dma_start(Wo[:, h], Wo_d[h * D:(h + 1) * D, :])
            bo_b = wpool.tile([P, E], f32)
            nc.sync.dma_start(bo_b[:], bo_d[0:1, :].to_broadcast((P, E)))

            gw1 = wpool.tile([P, 4, 128], f32)
            for c in range(4):
                nc.sync.dma_start(gw1[:, c], gw1_d[c * P:(c + 1) * P, :])
            cw1 = wpool.tile([P, 4, 64], f32)
            for c in range(4):
                nc.sync.dma_start(cw1[:, c], cw1_d[c * P:(c + 1) * P, :])
            gw2 = wpool.tile([P, 64], f32)
            nc.sync.dma_start(gw2[:], gw2_d[:, :])
            cw2 = wpool.tile([64, 32], f32)
            nc.sync.dma_start(cw2[:], cw2_d[:, :])
            gw3 = wpool.tile([64, 1], f32)
            nc.sync.dma_start(gw3[:], gw3_d[:, :])
            cw3 = wpool.tile([32, 1], f32)
            nc.sync.dma_start(cw3[:], cw3_d[:, :])
            smalls = {}
            for nm, d, pp in (("gb1", gb1_d, 128), ("gg", gg_d, 128),
                              ("gbe", gbe_d, 128), ("gb2", gb2_d, 64),
                              ("gb3", gb3_d, 1), ("cb1", cb1_d, 64),
                              ("cg", cg_d, 64), ("cbe", cbe_d, 64),
                              ("cb2", cb2_d, 32), ("cb3", cb3_d, 1)):
                t = wpool.tile([pp, 1], f32, tag=nm)
                nc.sync.dma_start(t[:], d[:, :])
                smalls[nm] = t
            pos_row = wpool.tile([1, R], f32)
            nc.sync.dma_start(pos_row[:], pos_d[:, :])
            tbl = wpool.tile([NK, 3 * NK], f32)
            nc.sync.dma_start(tbl[:], tbl_d[:, :])
            tau1 = wpool.tile([NK, 1], f32)
            nc.sync.dma_start(tau1[:], tau1_d[:, :])
            tau2 = wpool.tile([NK, 1], f32)
            nc.sync.dma_start(tau2[:], tau2_d[:, :])

            ident = wpool.tile([P, P], bf16)
            make_identity(nc, ident[:])
            ident_f = wpool.tile([P, P], f32)
            make_identity(nc, ident_f[:])
            ones1_20 = wpool.tile([1, NK], f32)
            nc.vector.memset(ones1_20[:], 1.0)
            ones20c = wpool.tile([NK, 1], bf16)
            nc.vector.memset(ones20c[:], 1.0)
            negC = wpool.tile([P, 1], f32)
            nc.vector.memset(negC[:], -CSHIFT)

            # ---------- gate MLP (feature-major layout: [feat, rows])
            def mlp_branch(w1, b1, g, be, w2, b2, w3, b3, f1, f2, tg):
                h1_ps = psA.tile([f1, R], f32, tag="m")
                for c in range(4):
                    nc.tensor.matmul(h1_ps[:], w1[:, c], xT[:, c],
                                     start=(c == 0), stop=(c == 3))
                h1 = pool.tile([f1, R], f32, tag="mh1")
                nc.vector.tensor_scalar(h1[:], h1_ps[:], b1[:], None,
                                        op0=A.add)
                sq = pool.tile([f1, R], f32, tag="msq")
                nc.vector.tensor_tensor(sq[:], h1[:], h1[:], op=A.mult)
                onesf = rowp.tile([f1, 1], f32, tag="mof")
                nc.vector.memset(onesf[:], 1.0)
                mu_ps = psA.tile([1, R], f32, tag="m")
                nc.tensor.matmul(mu_ps[:], onesf[:], h1[:],
                                 start=True, stop=True)
                s2_ps = psA.tile([1, R], f32, tag="m")
                nc.tensor.matmul(s2_ps[:], onesf[:], sq[:],
                                 start=True, stop=True)
                mu = rowp.tile([1, R], f32, tag="mmu")
                nc.vector.tensor_scalar(mu[:], mu_ps[:], 1.0 / f1, None,
                                        op0=A.mult)
                m2 = rowp.tile([1, R], f32, tag="mm2")
                nc.vector.tensor_scalar(m2[:], s2_ps[:], 1.0 / f1, None,
                                        op0=A.mult)
                var = rowp.tile([1, R], f32, tag="mvar")
                nc.vector.tensor_tensor(var[:], mu[:], mu[:], op=A.mult)
                nc.vector.tensor_tensor(var[:], m2[:], var[:], op=A.subtract)
                eps = rowp.tile([1, 1], f32, tag="meps")
                nc.vector.memset(eps[:], 1e-5)
                std = rowp.tile([1, R], f32, tag="mstd")
                nc.scalar.activation(std[:], var[:], AF.Sqrt, bias=eps[:])
                rstd = rowp.tile([1, R], f32, tag="mrstd")
                nc.vector.reciprocal(rstd[:], std[:])
                onesr = rowp.tile([1, f1], f32, tag="mor")
                nc.vector.memset(onesr[:], 1.0)
                mb_ps = psA.tile([f1, R], f32, tag="m")
                nc.tensor.matmul(mb_ps[:], onesr[:], mu[:],
                                 start=True, stop=True)
                rb_ps = psB.tile([f1, R], f32, tag="b")
                nc.tensor.matmul(rb_ps[:], onesr[:], rstd[:],
                                 start=True, stop=True)
                hc = pool.tile([f1, R], f32, tag="mhc")
                nc.vector.tensor_tensor(hc[:], h1[:], mb_ps[:], op=A.subtract)
                nc.vector.tensor_tensor(hc[:], hc[:], rb_ps[:], op=A.mult)
                hn = pool.tile([f1, R], f32, tag="mhn")
                nc.vector.tensor_scalar(hn[:], hc[:], g[:], be[:],
                                        op0=A.mult, op1=A.add)
                nc.vector.tensor_scalar(hn[:], hn[:], 0.0, None, op0=A.max)
                h2_ps = psA.tile([f2, R], f32, tag="m")
                nc.tensor.matmul(h2_ps[:], w2[:], hn[:], start=True, stop=True)
                h2 = pool.tile([f2, R], f32, tag="mh2")
                nc.vector.tensor_scalar(h2[:], h2_ps[:], b2[:], 0.0,
                                        op0=A.add, op1=A.max)
                h3_ps = psB.tile([1, R], f32, tag="b")
                nc.tensor.matmul(h3_ps[:], w3[:], h2[:], start=True, stop=True)
                sig = rowp.tile([1, R], f32, tag=f"{tg}sig")
                nc.scalar.activation(sig[:], h3_ps[:], AF.Sigmoid, bias=b3[:])
                return sig

            g3 = mlp_branch(gw1, smalls["gb1"], smalls["gg"], smalls["gbe"],
                            gw2, smalls["gb2"], gw3, smalls["gb3"],
                            128, 64, "g")
            c3 = mlp_branch(cw1, smalls["cb1"], smalls["cg"], smalls["cbe"],
                            cw2, smalls["cb2"], cw3, smalls["cb3"],
                            64, 32, "c")

            # y = 20 * ((0.7 g + 0.3 c) * pos), mirroring reference rounding
            c3s = rowp.tile([1, R], f32)
            nc.vector.tensor_scalar(c3s[:], c3[:], 0.3, None, op0=A.mult)
            y = rowp.tile([1, R], f32)
            nc.vector.scalar_tensor_tensor(out=y[:], in0=g3[:], scalar=0.7,
                                           in1=c3s[:], op0=A.mult, op1=A.add)
            nc.vector.tensor_tensor(y[:], y[:], pos_row[:], op=A.mult)
            nc.vector.tensor_scalar(y[:], y[:], 20.0, None, op0=A.mult)

            # staircase -> T -> one-hot, all [20, rows]
            yb_ps = psA.tile([NK, R], f32, tag="m")
            nc.tensor.matmul(yb_ps[:], ones1_20[:], y[:], start=True, stop=True)
            yb = rowp.tile([NK, R], f32)
            nc.vector.tensor_copy(yb[:], yb_ps[:])
            St = rowp.tile([NK, R], bf16)
            nc.vector.tensor_scalar(St[:], yb[:], tau1[:], None, op0=A.is_gt)
            T_ps = psB.tile([1, R], f32, tag="b")
            nc.tensor.matmul(T_ps[:], ones20c[:], St[:], start=True, stop=True)
            T_row = rowp.tile([1, R], f32)
            nc.vector.tensor_copy(T_row[:], T_ps[:])
            Tb_ps = psA.tile([NK, R], f32, tag="m")
            nc.tensor.matmul(Tb_ps[:], ones1_20[:], T_row[:],
                             start=True, stop=True)
            Tb = rowp.tile([NK, R], f32)
            nc.vector.tensor_copy(Tb[:], Tb_ps[:])
            Ot = rowp.tile([NK, R], f32)
            nc.vector.tensor_scalar(Ot[:], Tb[:], tau2[:], None, op0=A.is_equal)

            # per row-tile thresholds G [128, 60] (and negated, for ACT Sign)
            G = cpool.tile([P, 4, 3 * NK], f32)
            nG = cpool.tile([P, 4, 3 * NK], f32)
            for rt in range(4):
                g_ps = psB.tile([P, 3 * NK], f32, tag="b")
                nc.tensor.matmul(g_ps[:], Ot[:, rt * P:(rt + 1) * P], tbl[:],
                                 start=True, stop=True)
                nc.vector.tensor_copy(G[:, rt], g_ps[:])
                nc.vector.tensor_scalar(nG[:, rt], g_ps[:], -1.0, None,
                                        op0=A.mult)

            # ---------- QKV projections (fp32) + spike counts
            # k and v first (they feed the pair AllGather), q overlaps it.
            # compares: k on ACT (Sign), q/v on DVE (is_ge);
            # adds: q/v PE identity-accumulate, k DVE bf16 chain.
            cnt = {nm: cpool.tile([P, 4, E], bf16, tag=f"cnt_{nm}",
                                  name=f"cnt_{nm}")
                   for nm in ("q", "k", "v")}
            colbase = {"q": 0, "k": NK, "v": 2 * NK}
            kTl = cpool.tile([P, 4, R], bf16, tag="kTl")
            qA = cpool.tile([D + 1, H, R], bf16, tag="qA")

            def project(nm, rt):
                pj_ps = psA.tile([P, E], f32, tag="m", name="pj_ps")
                for c in range(4):
                    nc.tensor.matmul(pj_ps[:],
                                     xT[:, c, rt * P:(rt + 1) * P],
                                     Ws[nm][:, c],
                                     start=(c == 0), stop=(c == 3))
                t = pool.tile([P, E], f32, tag="pj_k" if nm == "k" else "pj_qv", name=f"pj_{nm}")
                nc.vector.tensor_copy(t[:], pj_ps[:])
                return t

            def counts_pe(nm, rt, pjt):
                """DVE compares + PE identity-accumulate."""
                cb = colbase[nm]
                acc_ps = psC.tile([P, E], f32, tag="acc", name="acc_ps")
                for k in range(NK):
                    ck = pool.tile([P, E], bf16, tag=f"ck{k % 2}", name="ck")
                    nc.vector.tensor_scalar(
                        ck[:], pjt[:], G[:, rt, cb + k:cb + k + 1],
                        None, op0=A.is_ge)
                    nc.tensor.matmul(acc_ps[:], ident[:], ck[:],
                                     start=(k == 0), stop=(k == NK - 1),
                                     skip_group_check=True)
                nc.scalar.copy(cnt[nm][:, rt], acc_ps[:])

            def counts_k(rt, pjt):
                """ACT Sign compares + DVE bf16 chain adds + affine fix."""
                cb = colbase["k"]
                kacc = None
                sks = []
                for k in range(NK):
                    sk = pool.tile([P, E], bf16, tag=f"sk{k % 2}", name="sk")
                    nc.scalar.sign(sk[:], pjt[:],
                                   bias=nG[:, rt, cb + k:cb + k + 1])
                    sks.append(sk)
                    if len(sks) == 2:
                        na = pool.tile([P, E], bf16, tag="ka", name="ka")
                        if kacc is None:
                            nc.vector.tensor_tensor(na[:], sks[0][:],
                                                    sks[1][:], op=A.add)
                        else:
                            nc.vector.tensor_tensor(na[:], kacc[:], sks[0][:],
                                                    op=A.add)
                            na2 = pool.tile([P, E], bf16, tag="kb", name="kb")
                            nc.vector.tensor_tensor(na2[:], na[:], sks[1][:],
                                                    op=A.add)
                            na = na2
                        kacc = na
                        sks = []
                nc.vector.tensor_scalar(cnt["k"][:, rt], kacc[:], 0.5, 10.0,
                                        op0=A.mult, op1=A.add)

            def transpose_k(rt):
                for ec in range(4):
                    t_ps = psB.tile([P, P], bf16, tag="b", name="t_ps")
                    nc.tensor.matmul(
                        t_ps[:], cnt["k"][:, rt, ec * P:(ec + 1) * P],
                        ident[:], is_transpose=True)
                    nc.scalar.copy(kTl[:, ec, rt * P:(rt + 1) * P], t_ps[:])

            def transpose_q(rt):
                for ec in range(4):
                    t_ps = psB.tile([P, P], bf16, tag="b", name="t_ps")
                    nc.tensor.matmul(
                        t_ps[:], cnt["q"][:, rt, ec * P:(ec + 1) * P],
                        ident[:], is_transpose=True)
                    nc.vector.tensor_copy(
                        qA[0:D, 2 * ec, rt * P:(rt + 1) * P], t_ps[0:D, :])
                    nc.vector.tensor_copy(
                        qA[0:D, 2 * ec + 1, rt * P:(rt + 1) * P],
                        t_ps[D:2 * D, :])

            snd = dpool.tile([2, 4, P, E], bf16)
            rcv = dpool.tile([2, 2, 4, P, E], bf16)
            # k and v counts first (collective inputs); send per-rt
            for rt in range(4):
                pk = project("k", rt)
                pv_ = project("v", rt)
                counts_k(rt, pk)
                counts_pe("v", rt, pv_)
                transpose_k(rt)
                nc.sync.dma_start(
                    snd[0, rt].rearrange("p (ec rc) -> p ec rc", ec=4, rc=P),
                    kTl[:, :, rt * P:(rt + 1) * P])
                nc.scalar.dma_start(snd[1, rt], cnt["v"][:, rt])

            nc.gpsimd.collective_compute(
                "AllGather", mybir.AluOpType.bypass,
                ins=[snd.opt()], outs=[rcv.opt()],
                replica_groups=[[0, 1], [2, 3], [4, 5], [6, 7]],
            )

            # q counts overlap the collective: 10 ACT signs (+-1) and
            # 10 DVE doubled indicators (0/2) PE-accumulated; count=(T+10)/2
            for rt in range(4):
                pq = project("q", rt)
                cb = colbase["q"]
                acc_ps = psC.tile([P, E], f32, tag="acc", name="acc_ps")
                for k in range(NK):
                    ck = pool.tile([P, E], bf16, tag=f"ck{k % 2}", name="ck")
                    if k < 10:
                        nc.scalar.sign(ck[:], pq[:],
                                       bias=nG[:, rt, cb + k:cb + k + 1])
                    else:
                        nc.vector.tensor_scalar(
                            ck[:], pq[:], G[:, rt, cb + k:cb + k + 1],
                            2.0, op0=A.is_ge, op1=A.mult)
                    nc.tensor.matmul(acc_ps[:], ident[:], ck[:],
                                     start=(k == 0), stop=(k == NK - 1),
                                     skip_group_check=True)
                nc.vector.tensor_scalar(cnt["q"][:, rt], acc_ps[:], 0.5, 5.0,
                                        op0=A.mult, op1=A.add)
                transpose_q(rt)

            # kA [65, H, cols]: memset ones, DMA count rows over it
            kA = cpool.tile([D + 1, H, S], bf16, tag="kA")
            nc.gpsimd.memset(kA[:], 1.0)
            for rank in range(2):
                for rt in range(4):
                    nc.sync.dma_start(
                        kA[0:D, :, rank * R + rt * P:
                           rank * R + (rt + 1) * P].rearrange(
                            "d (ec h2) rc -> d ec h2 rc", ec=4, h2=2),
                        rcv[rank, 0, rt].rearrange(
                            "(h2 d) (ec rc) -> d ec h2 rc",
                            h2=2, d=D, ec=4, rc=P))
            # v_aug [128, colchunk, head, 65]: same trick for the ones col
            v_aug = cpool.tile([P, 8, H, D + 1], bf16, tag="v_aug")
            nc.gpsimd.memset(v_aug[:], 1.0)
            for rank in range(2):
                for j in range(4):
                    eng = (nc.sync, nc.scalar, nc.gpsimd, nc.sync)[j]
                    eng.dma_start(
                        v_aug[:, rank * 4 + j, :, 0:D],
                        rcv[rank, 1, j].rearrange(
                            "p (h d) -> p h d", h=H, d=D))

            # ---------- shifted-bound aug row: -(q . ksum)/S per head
            ksum = cpool.tile([D, H, 1], f32, tag="ksum")
            ksum_bf = cpool.tile([D, H, 1], bf16, tag="ksum_bf")
            for h in range(H):
                nc.vector.reduce_sum(ksum[:, h], kA[0:D, h, :], axis=X)
                nc.vector.tensor_copy(ksum_bf[:, h], ksum[:, h])
                aug_ps = psB.tile([1, R], f32, tag="b")
                nc.tensor.matmul(aug_ps[:], ksum_bf[:, h], qA[0:D, h, :],
                                 start=True, stop=True)
                nc.scalar.activation(qA[D:D + 1, h, :], aug_ps[:],
                                     AF.Copy, scale=-1.0 / S)

            # ---------- attention: scores^T -> exp -> transposed PV
            # pv_T [65, rows] = v_aug.T @ w^T per head; row 64 is the softmax
            # denominator (from v_aug's ones column).
            UT = cpool.tile([D, H, R], bf16, tag="UT")
            den_hold = cpool.tile([D + 1, H, R], bf16, tag="den_hold")
            for h in range(H):
                w_h = []
                for pb in range(4):
                    sc_ps = psA.tile([P, 2, R], f32, tag="m", name="sc_ps")
                    for half in range(2):
                        cb_ = pb * 2 + half
                        nc.tensor.matmul(sc_ps[:, half],
                                         kA[:, h, cb_ * P:(cb_ + 1) * P],
                                         qA[:, h, :], start=True, stop=True,
                                         skip_group_check=True)
                    w_sb = pool.tile([P, 2, R], bf16, tag=f"w{pb}", bufs=1,
                                     name="w_sb")
                    nc.scalar.activation(w_sb[:], sc_ps[:], AF.Exp,
                                         scale=SCALE, bias=negC[:])
                    w_h.append(w_sb)
                pvt_ps = psB.tile([D + 1, R], f32, tag="pv", bufs=1)
                for cc in range(8):
                    nc.tensor.matmul(pvt_ps[:], v_aug[:, cc, h],
                                     w_h[cc // 2][:, cc % 2],
                                     start=(cc == 0), stop=(cc == 7),
                                     skip_group_check=True)
                nc.vector.tensor_copy(UT[:, h, :], pvt_ps[0:D, :])
                nc.vector.tensor_copy(den_hold[D:D + 1, h, :],
                                      pvt_ps[D:D + 1, :])

            # transpose denominators to row-major, reciprocal, back to rows
            recT = cpool.tile([P, 4, H], f32, tag="recT")
            for rt in range(4):
                dT = pool.tile([P, H], bf16, tag="dT", name="dT")
                for h in range(H):
                    m_ps = psB.tile([P, 1], bf16, tag="b", name="m_ps")
                    nc.tensor.matmul(
                        m_ps[:],
                        den_hold[D:D + 1, h, rt * P:(rt + 1) * P],
                        ident[D:D + 1, D:D + 1], is_transpose=True)
                    nc.vector.tensor_copy(dT[:, h:h + 1], m_ps[:])
                nc.vector.reciprocal(recT[:, rt, :], dT[:])
            rrow = cpool.tile([H, R], bf16, tag="rrow")
            for rt in range(4):
                r_ps = psB.tile([H, P], bf16, tag="b", name="r_ps")
                rT16 = pool.tile([P, H], bf16, tag="rT16", name="rT16")
                nc.vector.tensor_copy(rT16[:], recT[:, rt, :])
                nc.tensor.matmul(r_ps[:], rT16[:], ident[:],
                                 is_transpose=True)
                nc.vector.tensor_copy(rrow[:, rt * P:(rt + 1) * P], r_ps[:])
            rrow_d = dpool.tile([H, R], bf16)
            nc.sync.dma_start(rrow_d[:], rrow[:])
            recip_b = cpool.tile([D, H, R], bf16, tag="recip_b")
            for h in range(H):
                nc.sync.dma_start(recip_b[:, h, :],
                                  rrow_d[h:h + 1, :].to_broadcast((D, R)))
            nc.vector.tensor_tensor(UT[:], UT[:], recip_b[:], op=A.mult)

            # out = sum_h UT_h.T @ Wo[h-rows] + bo   (K=64 per head)
            for rt in range(4):
                o_ps = psA.tile([P, E], f32, tag="m", name="o_ps")
                for h in range(H):
                    nc.tensor.matmul(o_ps[:],
                                     UT[:, h, rt * P:(rt + 1) * P],
                                     Wo[:, h, :],
                                     start=(h == 0), stop=(h == H - 1))
                o_sb = pool.tile([P, E], f32, tag="o_sb", name="o_sb")
                nc.vector.tensor_tensor(o_sb[:], o_ps[:], bo_b[:], op=A.add)
                nc.sync.dma_start(out_d[rt * P:(rt + 1) * P, :], o_sb[:])

    nc.compile()
    return nc


# ------------------------------------------------------------------- driver
def kernel(**inputs) -> np.ndarray:
    import ml_dtypes
    global _compiled
    inp = {k: np.asarray(v) for k, v in inputs.items()}
    x = inp["x"].astype(np.float32)
    B = x.shape[0]

    thr_q = _build_thr_table(inp["alpha_q"], inp["beta_q"])
    thr_k = _build_thr_table(inp["alpha_k"], inp["beta_k"])
    thr_v = _build_thr_table(inp["alpha_v"], inp["beta_v"])
    tbl_all = np.concatenate([thr_q, thr_k, thr_v], axis=1)  # [20, 60]

    pos_full = np.linspace(0.8, 1.2, S, dtype=np.float32)
    tau1 = np.array([-1.0] + [float(j) for j in range(1, NK)],
                    np.float32).reshape(NK, 1)
    tau2 = np.arange(1, NK + 1, dtype=np.float32).reshape(NK, 1)
    Wo_s16 = (inp["Wo"].astype(np.float64) / T_MAX).astype(
        np.float32).astype(ml_dtypes.bfloat16)

    def col(a):
        return np.ascontiguousarray(np.asarray(a, np.float32).reshape(-1, 1))

    common = {
        "Wq": np.ascontiguousarray(inp["Wq"].astype(np.float32)),
        "Wk": np.ascontiguousarray(inp["Wk"].astype(np.float32)),
        "Wv": np.ascontiguousarray(inp["Wv"].astype(np.float32)),
        "Wo_s": np.ascontiguousarray(Wo_s16),
        "bo_row": np.ascontiguousarray(
            inp["bo"].astype(np.float32).reshape(1, E)),
        "gW1": np.ascontiguousarray(inp["gW1"].astype(np.float32)),
        "gb1": col(inp["gb1"]), "gg": col(inp["gg"]), "gbe": col(inp["gbe"]),
        "gW2": np.ascontiguousarray(inp["gW2"].astype(np.float32)),
        "gb2": col(inp["gb2"]),
        "gW3": np.ascontiguousarray(inp["gW3"].astype(np.float32)),
        "gb3": col(inp["gb3"]),
        "cW1": np.ascontiguousarray(inp["cW1"].astype(np.float32)),
        "cb1": col(inp["cb1"]), "cg": col(inp["cg"]), "cbe": col(inp["cbe"]),
        "cW2": np.ascontiguousarray(inp["cW2"].astype(np.float32)),
        "cb2": col(inp["cb2"]),
        "cW3": np.ascontiguousarray(inp["cW3"].astype(np.float32)),
        "cb3": col(inp["cb3"]),
        "tbl_all": np.ascontiguousarray(tbl_all),
        "tau1": tau1, "tau2": tau2,
    }

    in_maps = []
    for c in range(8):
        b, half = c // 2, c % 2
        rows = slice(half * R, half * R + R)
        m = dict(common)
        m["xT"] = np.ascontiguousarray(x[b, rows].T)
        m["pos_row"] = np.ascontiguousarray(pos_full[rows].reshape(1, R))
        in_maps.append(m)

    if _compiled is None:
        _compiled = _build_program()
    nc = _compiled

    res = run_bass_kernel_spmd(nc, in_maps, core_ids=list(range(8)))

    out = np.zeros((B, S, E), np.float32)
    for c in range(8):
        b, half = c // 2, c % 2
        out[b, half * R:(half + 1) * R, :] = res.results[c]["out"]
    return out


# revision 33
# speedup vs baseline: 1.0067x; 1.0067x over previous
"""AdaptiveSpikingAttention on 8 TRN2 NeuronCores (Bass/Tile).

Sharding: the 4096 (batch, seq) rows are split across 8 cores — core c owns
batch c//2, half c%2 (512 rows). Projections, gate MLPs and spike counting
are row-local; the two cores of a batch exchange k/v spike counts with a
pair AllGather before the attention.

Key transform: the 20-step LIF spike recurrence acc(x, T) is a monotone
step function of x whose <=20 jump points depend only on (alpha, beta, T).
The jump points are bisected on the host from the scalar parameters; on
device each element needs 20 compares against per-row thresholds instead
of a sequential 20-step recurrence.

Softmax: scores only ever exist transposed ([col, row]); the row bound
M_i = scale*(q_i . kmean) + C is folded into the score matmul as an extra
contraction row. row-max >= row-mean keeps the denominator well away from
underflow, and C centers the exp arguments in fp32 range.
"""

import sys
import numpy as np

sys.path.insert(0, "/opt/trn_rl_repo")

import concourse.bass as bass
import concourse.bacc as bacc
import concourse.tile as tile
import concourse.mybir as mybir
from concourse.bass_utils import run_bass_kernel_spmd
from concourse.masks import make_identity

f32 = mybir.dt.float32
bf16 = mybir.dt.bfloat16
P = 128
R = 512           # rows per core
E = 512
H, D = 8, 64
S = 1024
NK = 20           # thresholds per tensor
T_MAX = 20
CSHIFT = 114.0    # exp-range centering constant
SCALE = float(D) ** -0.5

_compiled = None


# ----------------------------------------------------------------- host math
def _build_thr_table(alpha, beta):
    """thr[T-1, k-1]: smallest f32 x with count(x, T) >= k (64.0 if never)."""
    alpha = np.float32(alpha)
    beta = np.float32(beta)

    def counts(xs, T):
        xs = xs.astype(np.float32)
        v = np.zeros_like(xs)
        i = np.zeros_like(xs)
        acc = np.zeros_like(xs)
        for t in range(T_MAX):
            a = np.float32(1.0) if t < T else np.float32(0.0)
            i = alpha * i + xs * a
            v = beta * v + i
            s = (v >= 1.0).astype(np.float32)
            v = v * (1.0 - s)
            acc = acc + s * a
        return acc

    thr = np.full((T_MAX, T_MAX), np.float32(64.0), np.float32)
    for T in range(1, T_MAX + 1):
        los = np.full(T, -3, np.float32)
        his = np.full(T, 6, np.float32)
        ks = np.arange(1, T + 1)
        for _ in range(60):
            mids = ((los.astype(np.float64) + his) / 2).astype(np.float32)
            ge = counts(mids, T) >= ks
            his = np.where(ge, mids, his)
            los = np.where(ge, los, mids)
        thr[T - 1, :T] = his
    return thr


# -------------------------------------------------------------- device build
def _build_program():
    nc = bacc.Bacc("TRN2", target_bir_lowering=False, debug=False,
                   enable_asserts=True, num_devices=8)
    A = mybir.AluOpType
    AF = mybir.ActivationFunctionType
    X = mybir.AxisListType.X

    def dram(name, shape, dt=f32, kind="ExternalInput"):
        return nc.dram_tensor(name, shape, dt, kind=kind)

    xT_d = dram("xT", [E, R])
    Wq_d = dram("Wq", [E, E])
    Wk_d = dram("Wk", [E, E])
    Wv_d = dram("Wv", [E, E])
    Wo_d = dram("Wo_s", [E, E], bf16)
    bo_d = dram("bo_row", [1, E])
    gw1_d = dram("gW1", [E, 128]); gb1_d = dram("gb1", [128, 1])
    gg_d = dram("gg", [128, 1]); gbe_d = dram("gbe", [128, 1])
    gw2_d = dram("gW2", [128, 64]); gb2_d = dram("gb2", [64, 1])
    gw3_d = dram("gW3", [64, 1]); gb3_d = dram("gb3", [1, 1])
    cw1_d = dram("cW1", [E, 64]); cb1_d = dram("cb1", [64, 1])
    cg_d = dram("cg", [64, 1]); cbe_d = dram("cbe", [64, 1])
    cw2_d = dram("cW2", [64, 32]); cb2_d = dram("cb2", [32, 1])
    cw3_d = dram("cW3", [32, 1]); cb3_d = dram("cb3", [1, 1])
    pos_d = dram("pos_row", [1, R])
    tbl_d = dram("tbl_all", [NK, 3 * NK])
    tau1_d = dram("tau1", [NK, 1])
    tau2_d = dram("tau2", [NK, 1])
    out_d = dram("out", [R, E], kind="ExternalOutput")

    with tile.TileContext(nc) as tc:
        with (
            tc.tile_pool(name="w", bufs=1) as wpool,
            tc.tile_pool(name="sb", bufs=2) as pool,
            tc.tile_pool(name="row", bufs=1) as rowp,
            tc.tile_pool(name="cnt", bufs=1) as cpool,
            tc.tile_pool(name="psA", bufs=2, space="PSUM") as psA,
            tc.tile_pool(name="psB", bufs=2, space="PSUM") as psB,
            tc.tile_pool(name="psC", bufs=1, space="PSUM") as psC,
            tc.tile_pool(name="dram", bufs=1, space="DRAM") as dpool,
        ):
            # ---------- load everything
            xT = wpool.tile([P, 4, R], f32)
            for c in range(4):
                nc.sync.dma_start(xT[:, c], xT_d[c * P:(c + 1) * P, :])
            Ws = {}
            for qi, (nm, d) in enumerate((("q", Wq_d), ("k", Wk_d),
                                          ("v", Wv_d))):
                W = wpool.tile([P, 4, E], f32, tag=f"W{nm}")
                eng = (nc.gpsimd, nc.scalar, nc.gpsimd)[qi]
                for c in range(4):
                    eng.dma_start(W[:, c], d[c * P:(c + 1) * P, :])
                Ws[nm] = W
            Wo = wpool.tile([D, H, E], bf16)
            for h in range(H):
                nc.scalar.dma_start(Wo[:, h], Wo_d[h * D:(h + 1) * D, :])
            bo_b = wpool.tile([P, E], f32)
            nc.sync.dma_start(bo_b[:], bo_d[0:1, :].to_broadcast((P, E)))

            gw1 = wpool.tile([P, 4, 128], f32)
            for c in range(4):
                nc.sync.dma_start(gw1[:, c], gw1_d[c * P:(c + 1) * P, :])
            cw1 = wpool.tile([P, 4, 64], f32)
            for c in range(4):
                nc.sync.dma_start(cw1[:, c], cw1_d[c * P:(c + 1) * P, :])
            gw2 = wpool.tile([P, 64], f32)
            nc.sync.dma_start(gw2[:], gw2_d[:, :])
            cw2 = wpool.tile([64, 32], f32)
            nc.sync.dma_start(cw2[:], cw2_d[:, :])
            gw3 = wpool.tile([64, 1], f32)
            nc.sync.dma_start(gw3[:], gw3_d[:, :])
            cw3 = wpool.tile([32, 1], f32)
            nc.sync.dma_start(cw3[:], cw3_d[:, :])
            smalls = {}
            for nm, d, pp in (("gb1", gb1_d, 128), ("gg", gg_d, 128),
                              ("gbe", gbe_d, 128), ("gb2", gb2_d, 64),
                              ("gb3", gb3_d, 1), ("cb1", cb1_d, 64),
                              ("cg", cg_d, 64), ("cbe", cbe_d, 64),
                              ("cb2", cb2_d, 32), ("cb3", cb3_d, 1)):
                t = wpool.tile([pp, 1], f32, tag=nm)
                nc.sync.dma_start(t[:], d[:, :])
                smalls[nm] = t
            pos_row = wpool.tile([1, R], f32)
            nc.sync.dma_start(pos_row[:], pos_d[:, :])
            tbl = wpool.tile([NK, 3 * NK], f32)
            nc.sync.dma_start(tbl[:], tbl_d[:, :])
            tau1 = wpool.tile([NK, 1], f32)
            nc.sync.dma_start(tau1[:], tau1_d[:, :])
            tau2 = wpool.tile([NK, 1], f32)
            nc.sync.dma_start(tau2[:], tau2_d[:, :])

            ident = wpool.tile([P, P], bf16)
            make_identity(nc, ident[:])
            ident_f = wpool.tile([P, P], f32)
            make_identity(nc, ident_f[:])
            ones1_20 = wpool.tile([1, NK], f32)
            nc.vector.memset(ones1_20[:], 1.0)
            ones20c = wpool.tile([NK, 1], bf16)
            nc.vector.memset(ones20c[:], 1.0)
            negC = wpool.tile([P, 1], f32)
            nc.vector.memset(negC[:], -CSHIFT)

            # ---------- gate MLP (feature-major layout: [feat, rows])
            def mlp_branch(w1, b1, g, be, w2, b2, w3, b3, f1, f2, tg):
                h1_ps = psA.tile([f1, R], f32, tag="m")
                for c in range(4):
                    nc.tensor.matmul(h1_ps[:], w1[:, c], xT[:, c],
                                     start=(c == 0), stop=(c == 3))
                h1 = pool.tile([f1, R], f32, tag="mh1")
                nc.vector.tensor_scalar(h1[:], h1_ps[:], b1[:], None,
                                        op0=A.add)
                sq = pool.tile([f1, R], f32, tag="msq")
                nc.vector.tensor_tensor(sq[:], h1[:], h1[:], op=A.mult)
                onesf = rowp.tile([f1, 1], f32, tag="mof")
                nc.vector.memset(onesf[:], 1.0)
                mu_ps = psA.tile([1, R], f32, tag="m")
                nc.tensor.matmul(mu_ps[:], onesf[:], h1[:],
                                 start=True, stop=True)
                s2_ps = psA.tile([1, R], f32, tag="m")
                nc.tensor.matmul(s2_ps[:], onesf[:], sq[:],
                                 start=True, stop=True)
                mu = rowp.tile([1, R], f32, tag="mmu")
                nc.vector.tensor_scalar(mu[:], mu_ps[:], 1.0 / f1, None,
                                        op0=A.mult)
                m2 = rowp.tile([1, R], f32, tag="mm2")
                nc.vector.tensor_scalar(m2[:], s2_ps[:], 1.0 / f1, None,
                                        op0=A.mult)
                var = rowp.tile([1, R], f32, tag="mvar")
                nc.vector.tensor_tensor(var[:], mu[:], mu[:], op=A.mult)
                nc.vector.tensor_tensor(var[:], m2[:], var[:], op=A.subtract)
                eps = rowp.tile([1, 1], f32, tag="meps")
                nc.vector.memset(eps[:], 1e-5)
                std = rowp.tile([1, R], f32, tag="mstd")
                nc.scalar.activation(std[:], var[:], AF.Sqrt, bias=eps[:])
                rstd = rowp.tile([1, R], f32, tag="mrstd")
                nc.vector.reciprocal(rstd[:], std[:])
                onesr = rowp.tile([1, f1], f32, tag="mor")
                nc.vector.memset(onesr[:], 1.0)
                mb_ps = psA.tile([f1, R], f32, tag="m")
                nc.tensor.matmul(mb_ps[:], onesr[:], mu[:],
                                 start=True, stop=True)
                rb_ps = psB.tile([f1, R], f32, tag="b", bufs=1)
                nc.tensor.matmul(rb_ps[:], onesr[:], rstd[:],
                                 start=True, stop=True)
                hc = pool.tile([f1, R], f32, tag="mhc")
                nc.vector.tensor_tensor(hc[:], h1[:], mb_ps[:], op=A.subtract)
                nc.vector.tensor_tensor(hc[:], hc[:], rb_ps[:], op=A.mult)
                hn = pool.tile([f1, R], f32, tag="mhn")
                nc.vector.tensor_scalar(hn[:], hc[:], g[:], be[:],
                                        op0=A.mult, op1=A.add)
                nc.vector.tensor_scalar(hn[:], hn[:], 0.0, None, op0=A.max)
                h2_ps = psA.tile([f2, R], f32, tag="m")
                nc.tensor.matmul(h2_ps[:], w2[:], hn[:], start=True, stop=True)
                h2 = pool.tile([f2, R], f32, tag="mh2")
                nc.vector.tensor_scalar(h2[:], h2_ps[:], b2[:], 0.0,
                                        op0=A.add, op1=A.max)
                h3_ps = psB.tile([1, R], f32, tag="b", bufs=1)
                nc.tensor.matmul(h3_ps[:], w3[:], h2[:], start=True, stop=True)
                sig = rowp.tile([1, R], f32, tag=f"{tg}sig")
                nc.scalar.activation(sig[:], h3_ps[:], AF.Sigmoid, bias=b3[:])
                return sig

            g3 = mlp_branch(gw1, smalls["gb1"], smalls["gg"], smalls["gbe"],
                            gw2, smalls["gb2"], gw3, smalls["gb3"],
                            128, 64, "g")
            c3 = mlp_branch(cw1, smalls["cb1"], smalls["cg"], smalls["cbe"],
                            cw2, smalls["cb2"], cw3, smalls["cb3"],
                            64, 32, "c")

            # y = 20 * ((0.7 g + 0.3 c) * pos), mirroring reference rounding
            c3s = rowp.tile([1, R], f32)
            nc.vector.tensor_scalar(c3s[:], c3[:], 0.3, None, op0=A.mult)
            y = rowp.tile([1, R], f32)
            nc.vector.scalar_tensor_tensor(out=y[:], in0=g3[:], scalar=0.7,
                                           in1=c3s[:], op0=A.mult, op1=A.add)
            nc.vector.tensor_tensor(y[:], y[:], pos_row[:], op=A.mult)
            nc.vector.tensor_scalar(y[:], y[:], 20.0, None, op0=A.mult)

            # staircase -> T -> one-hot, all [20, rows]
            yb_ps = psA.tile([NK, R], f32, tag="m")
            nc.tensor.matmul(yb_ps[:], ones1_20[:], y[:], start=True, stop=True)
            yb = rowp.tile([NK, R], f32)
            nc.vector.tensor_copy(yb[:], yb_ps[:])
            St = rowp.tile([NK, R], bf16)
            nc.vector.tensor_scalar(St[:], yb[:], tau1[:], None, op0=A.is_gt)
            T_ps = psB.tile([1, R], f32, tag="b", bufs=1)
            nc.tensor.matmul(T_ps[:], ones20c[:], St[:], start=True, stop=True)
            T_row = rowp.tile([1, R], f32)
            nc.vector.tensor_copy(T_row[:], T_ps[:])
            Tb_ps = psA.tile([NK, R], f32, tag="m")
            nc.tensor.matmul(Tb_ps[:], ones1_20[:], T_row[:],
                             start=True, stop=True)
            Tb = rowp.tile([NK, R], f32)
            nc.vector.tensor_copy(Tb[:], Tb_ps[:])
            Ot = rowp.tile([NK, R], f32)
            nc.vector.tensor_scalar(Ot[:], Tb[:], tau2[:], None, op0=A.is_equal)

            # per row-tile thresholds G [128, 60] (and negated, for ACT Sign)
            G = cpool.tile([P, 4, 3 * NK], f32)
            nG = cpool.tile([P, 4, 3 * NK], f32)
            for rt in range(4):
                g_ps = psB.tile([P, 3 * NK], f32, tag="b", bufs=1)
                nc.tensor.matmul(g_ps[:], Ot[:, rt * P:(rt + 1) * P], tbl[:],
                                 start=True, stop=True)
                nc.vector.tensor_copy(G[:, rt], g_ps[:])
                nc.vector.tensor_scalar(nG[:, rt], g_ps[:], -1.0, None,
                                        op0=A.mult)

            # ---------- QKV projections (fp32) + spike counts
            # k and v first (they feed the pair AllGather), q overlaps it.
            # compares: k on ACT (Sign), q/v on DVE (is_ge);
            # adds: q/v PE identity-accumulate, k DVE bf16 chain.
            cnt = {nm: cpool.tile([P, 4, E], bf16, tag=f"cnt_{nm}",
                                  name=f"cnt_{nm}")
                   for nm in ("q", "k", "v")}
            colbase = {"q": 0, "k": NK, "v": 2 * NK}
            kTl = cpool.tile([P, 4, R], bf16, tag="kTl")
            qA = cpool.tile([D + 1, H, R], bf16, tag="qA")

            def project(nm, rt):
                pj_ps = psA.tile([P, E], f32, tag="m", name="pj_ps")
                for c in range(4):
                    nc.tensor.matmul(pj_ps[:],
                                     xT[:, c, rt * P:(rt + 1) * P],
                                     Ws[nm][:, c],
                                     start=(c == 0), stop=(c == 3))
                t = pool.tile([P, E], f32, tag="pj_k" if nm == "k" else "pj_qv", name=f"pj_{nm}")
                nc.vector.tensor_copy(t[:], pj_ps[:])
                return t

            def counts_pe(nm, rt, pjt):
                """DVE compares + PE identity-accumulate."""
                cb = colbase[nm]
                acc_ps = psC.tile([P, E], f32, tag="acc", name="acc_ps")
                for k in range(NK):
                    ck = pool.tile([P, E], bf16, tag=f"ck{k % 2}", name="ck")
                    nc.vector.tensor_scalar(
                        ck[:], pjt[:], G[:, rt, cb + k:cb + k + 1],
                        None, op0=A.is_ge)
                    nc.tensor.matmul(acc_ps[:], ident[:], ck[:],
                                     start=(k == 0), stop=(k == NK - 1),
                                     skip_group_check=True)
                nc.scalar.copy(cnt[nm][:, rt], acc_ps[:])

            def counts_k(rt, pjt):
                """ACT Sign compares + DVE bf16 chain adds + affine fix."""
                cb = colbase["k"]
                kacc = None
                sks = []
                for k in range(NK):
                    sk = pool.tile([P, E], bf16, tag=f"sk{k % 2}", name="sk")
                    nc.scalar.sign(sk[:], pjt[:],
                                   bias=nG[:, rt, cb + k:cb + k + 1])
                    sks.append(sk)
                    if len(sks) == 2:
                        na = pool.tile([P, E], bf16, tag="ka", name="ka")
                        if kacc is None:
                            nc.vector.tensor_tensor(na[:], sks[0][:],
                                                    sks[1][:], op=A.add)
                        else:
                            nc.vector.tensor_tensor(na[:], kacc[:], sks[0][:],
                                                    op=A.add)
                            na2 = pool.tile([P, E], bf16, tag="kb", name="kb")
                            nc.vector.tensor_tensor(na2[:], na[:], sks[1][:],
                                                    op=A.add)
                            na = na2
                        kacc = na
                        sks = []
                nc.vector.tensor_scalar(cnt["k"][:, rt], kacc[:], 0.5, 10.0,
                                        op0=A.mult, op1=A.add)

            def transpose_k(rt):
                for ec in range(4):
                    t_ps = psB.tile([P, P], bf16, tag="b", bufs=1, name="t_ps")
                    nc.tensor.matmul(
                        t_ps[:], cnt["k"][:, rt, ec * P:(ec + 1) * P],
                        ident[:], is_transpose=True)
                    nc.scalar.copy(kTl[:, ec, rt * P:(rt + 1) * P], t_ps[:])

            def transpose_q(rt):
                for ec in range(4):
                    t_ps = psB.tile([P, P], bf16, tag="b", bufs=1, name="t_ps")
                    nc.tensor.matmul(
                        t_ps[:], cnt["q"][:, rt, ec * P:(ec + 1) * P],
                        ident[:], is_transpose=True)
                    nc.vector.tensor_copy(
                        qA[0:D, 2 * ec, rt * P:(rt + 1) * P], t_ps[0:D, :])
                    nc.vector.tensor_copy(
                        qA[0:D, 2 * ec + 1, rt * P:(rt + 1) * P],
                        t_ps[D:2 * D, :])

            snd = dpool.tile([2, 4, P, E], bf16)
            rcv = dpool.tile([2, 2, 4, P, E], bf16)
            # k and v counts first (collective inputs); send per-rt
            for rt in range(4):
                pk = project("k", rt)
                pv_ = project("v", rt)
                counts_k(rt, pk)
                counts_pe("v", rt, pv_)
                transpose_k(rt)
                nc.sync.dma_start(
                    snd[0, rt].rearrange("p (ec rc) -> p ec rc", ec=4, rc=P),
                    kTl[:, :, rt * P:(rt + 1) * P])
                nc.scalar.dma_start(snd[1, rt], cnt["v"][:, rt])

            nc.gpsimd.collective_compute(
                "AllGather", mybir.AluOpType.bypass,
                ins=[snd.opt()], outs=[rcv.opt()],
                replica_groups=[[0, 1], [2, 3], [4, 5], [6, 7]],
            )

            # q counts overlap the collective: 10 ACT signs (+-1) and
            # 10 DVE doubled indicators (0/2) PE-accumulated; count=(T+10)/2
            for rt in range(4):
                pq = project("q", rt)
                cb = colbase["q"]
                acc_ps = psC.tile([P, E], f32, tag="acc", name="acc_ps")
                for k in range(NK):
                    ck = pool.tile([P, E], bf16, tag=f"ck{k % 2}", name="ck")
                    if k < 10:
                        nc.scalar.sign(ck[:], pq[:],
                                       bias=nG[:, rt, cb + k:cb + k + 1])
                    else:
                        nc.vector.tensor_scalar(
                            ck[:], pq[:], G[:, rt, cb + k:cb + k + 1],
                            2.0, op0=A.is_ge, op1=A.mult)
                    nc.tensor.matmul(acc_ps[:], ident[:], ck[:],
                                     start=(k == 0), stop=(k == NK - 1),
                                     skip_group_check=True)
                nc.vector.tensor_scalar(cnt["q"][:, rt], acc_ps[:], 0.5, 5.0,
                                        op0=A.mult, op1=A.add)
                transpose_q(rt)

            # kA [65, H, cols]: memset ones, DMA count rows over it
            kA = cpool.tile([D + 1, H, S], bf16, tag="kA")
            nc.gpsimd.memset(kA[:], 1.0)
            for rank in range(2):
                for rt in range(4):
                    nc.sync.dma_start(
                        kA[0:D, :, rank * R + rt * P:
                           rank * R + (rt + 1) * P].rearrange(
                            "d (ec h2) rc -> d ec h2 rc", ec=4, h2=2),
                        rcv[rank, 0, rt].rearrange(
                            "(h2 d) (ec rc) -> d ec h2 rc",
                            h2=2, d=D, ec=4, rc=P))
            # v_aug [128, colchunk, head, 65]: same trick for the ones col
            v_aug = cpool.tile([P, 8, H, D + 1], bf16, tag="v_aug")
            nc.gpsimd.memset(v_aug[:], 1.0)
            for rank in range(2):
                for j in range(4):
                    eng = (nc.sync, nc.scalar, nc.gpsimd, nc.sync)[j]
                    eng.dma_start(
                        v_aug[:, rank * 4 + j, :, 0:D],
                        rcv[rank, 1, j].rearrange(
                            "p (h d) -> p h d", h=H, d=D))

            # ---------- shifted-bound aug row: -(q . ksum)/S per head
            ksum = cpool.tile([D, H, 1], f32, tag="ksum")
            ksum_bf = cpool.tile([D, H, 1], bf16, tag="ksum_bf")
            for h in range(H):
                nc.vector.reduce_sum(ksum[:, h], kA[0:D, h, :], axis=X)
                nc.vector.tensor_copy(ksum_bf[:, h], ksum[:, h])
                aug_ps = psB.tile([1, R], f32, tag="b", bufs=1)
                nc.tensor.matmul(aug_ps[:], ksum_bf[:, h], qA[0:D, h, :],
                                 start=True, stop=True)
                nc.scalar.activation(qA[D:D + 1, h, :], aug_ps[:],
                                     AF.Copy, scale=-1.0 / S)

            # ---------- attention: scores^T -> exp -> transposed PV
            # pv_T [65, rows] = v_aug.T @ w^T per head; row 64 is the softmax
            # denominator (from v_aug's ones column).
            UT = cpool.tile([D, H, R], bf16, tag="UT")
            den_hold = cpool.tile([D + 1, H, R], bf16, tag="den_hold")
            for h in range(H):
                w_h = []
                for pb in range(4):
                    sc_ps = psA.tile([P, 2, R], f32, tag="m", name="sc_ps")
                    for half in range(2):
                        cb_ = pb * 2 + half
                        nc.tensor.matmul(sc_ps[:, half],
                                         kA[:, h, cb_ * P:(cb_ + 1) * P],
                                         qA[:, h, :], start=True, stop=True,
                                         skip_group_check=True)
                    w_sb = pool.tile([P, 2, R], bf16, tag=f"w{pb}_{h % 2}", bufs=1,
                                     name="w_sb")
                    nc.scalar.activation(w_sb[:], sc_ps[:], AF.Exp,
                                         scale=SCALE, bias=negC[:])
                    w_h.append(w_sb)
                pvt_ps = psB.tile([D + 1, R], f32, tag="pv", bufs=2)
                for cc in range(8):
                    nc.tensor.matmul(pvt_ps[:], v_aug[:, cc, h],
                                     w_h[cc // 2][:, cc % 2],
                                     start=(cc == 0), stop=(cc == 7),
                                     skip_group_check=True)
                nc.vector.tensor_copy(UT[:, h, :], pvt_ps[0:D, :])
                nc.vector.tensor_copy(den_hold[D:D + 1, h, :],
                                      pvt_ps[D:D + 1, :])

            # transpose denominators to row-major, reciprocal, back to rows
            recT = cpool.tile([P, 4, H], f32, tag="recT")
            for rt in range(4):
                dT = pool.tile([P, H], bf16, tag="dT", name="dT")
                for h in range(H):
                    m_ps = psB.tile([P, 1], bf16, tag="b", bufs=1, name="m_ps")
                    nc.tensor.matmul(
                        m_ps[:],
                        den_hold[D:D + 1, h, rt * P:(rt + 1) * P],
                        ident[D:D + 1, D:D + 1], is_transpose=True)
                    nc.vector.tensor_copy(dT[:, h:h + 1], m_ps[:])
                nc.vector.reciprocal(recT[:, rt, :], dT[:])
            rrow = cpool.tile([H, R], bf16, tag="rrow")
            for rt in range(4):
                r_ps = psB.tile([H, P], bf16, tag="b", bufs=1, name="r_ps")
                rT16 = pool.tile([P, H], bf16, tag="rT16", name="rT16")
                nc.vector.tensor_copy(rT16[:], recT[:, rt, :])
                nc.tensor.matmul(r_ps[:], rT16[:], ident[:],
                                 is_transpose=True)
                nc.vector.tensor_copy(rrow[:, rt * P:(rt + 1) * P], r_ps[:])
            rrow_d = dpool.tile([H, R], bf16)
            nc.sync.dma_start(rrow_d[:], rrow[:])
            for h in range(H):
                rb = pool.tile([D, R], bf16, tag=f"rb{h % 2}", bufs=1,
                               name="rb")
                nc.sync.dma_start(rb[:],
                                  rrow_d[h:h + 1, :].to_broadcast((D, R)))
                nc.vector.tensor_tensor(UT[:, h, :], UT[:, h, :], rb[:],
                                        op=A.mult)

            # out = sum_h UT_h.T @ Wo[h-rows] + bo   (K=64 per head)
            for rt in range(4):
                o_ps = psA.tile([P, E], f32, tag="m", name="o_ps")
                for h in range(H):
                    nc.tensor.matmul(o_ps[:],
                                     UT[:, h, rt * P:(rt + 1) * P],
                                     Wo[:, h, :],
                                     start=(h == 0), stop=(h == H - 1))
                o_sb = pool.tile([P, E], f32, tag="o_sb", name="o_sb")
                nc.vector.tensor_tensor(o_sb[:], o_ps[:], bo_b[:], op=A.add)
                nc.sync.dma_start(out_d[rt * P:(rt + 1) * P, :], o_sb[:])

    nc.compile()
    return nc


# ------------------------------------------------------------------- driver
def kernel(**inputs) -> np.ndarray:
    import ml_dtypes
    global _compiled
    inp = {k: np.asarray(v) for k, v in inputs.items()}
    x = inp["x"].astype(np.float32)
    B = x.shape[0]

    thr_q = _build_thr_table(inp["alpha_q"], inp["beta_q"])
    thr_k = _build_thr_table(inp["alpha_k"], inp["beta_k"])
    thr_v = _build_thr_table(inp["alpha_v"], inp["beta_v"])
    tbl_all = np.concatenate([thr_q, thr_k, thr_v], axis=1)  # [20, 60]

    pos_full = np.linspace(0.8, 1.2, S, dtype=np.float32)
    tau1 = np.array([-1.0] + [float(j) for j in range(1, NK)],
                    np.float32).reshape(NK, 1)
    tau2 = np.arange(1, NK + 1, dtype=np.float32).reshape(NK, 1)
    Wo_s16 = (inp["Wo"].astype(np.float64) / T_MAX).astype(
        np.float32).astype(ml_dtypes.bfloat16)

    def col(a):
        return np.ascontiguousarray(np.asarray(a, np.float32).reshape(-1, 1))

    common = {
        "Wq": np.ascontiguousarray(inp["Wq"].astype(np.float32)),
        "Wk": np.ascontiguousarray(inp["Wk"].astype(np.float32)),
        "Wv": np.ascontiguousarray(inp["Wv"].astype(np.float32)),
        "Wo_s": np.ascontiguousarray(Wo_s16),
        "bo_row": np.ascontiguousarray(
            inp["bo"].astype(np.float32).reshape(1, E)),
        "gW1": np.ascontiguousarray(inp["gW1"].astype(np.float32)),
        "gb1": col(inp["gb1"]), "gg": col(inp["gg"]), "gbe": col(inp["gbe"]),
        "gW2": np.ascontiguousarray(inp["gW2"].astype(np.float32)),
        "gb2": col(inp["gb2"]),
        "gW3": np.ascontiguousarray(inp["gW3"].astype(np.float32)),
        "gb3": col(inp["gb3"]),
        "cW1": np.ascontiguousarray(inp["cW1"].astype(np.float32)),
        "cb1": col(inp["cb1"]), "cg": col(inp["cg"]), "cbe": col(inp["cbe"]),
        "cW2": np.ascontiguousarray(inp["cW2"].astype(np.float32)),
        "cb2": col(inp["cb2"]),
        "cW3": np.ascontiguousarray(inp["cW3"].astype(np.float32)),
        "cb3": col(inp["cb3"]),
        "tbl_all": np.ascontiguousarray(tbl_all),
        "tau1": tau1, "tau2": tau2,
    }

    in_maps = []
    for c in range(8):
        b, half = c // 2, c % 2
        rows = slice(half * R, half * R + R)
        m = dict(common)
        m["xT"] = np.ascontiguousarray(x[b, rows].T)
        m["pos_row"] = np.ascontiguousarray(pos_full[rows].reshape(1, R))
        in_maps.append(m)

    if _compiled is None:
        _compiled = _build_program()
    nc = _compiled

    res = run_bass_kernel_spmd(nc, in_maps, core_ids=list(range(8)))

    out = np.zeros((B, S, E), np.float32)
    for c in range(8):
        b, half = c // 2, c % 2
        out[b, half * R:(half + 1) * R, :] = res.results[c]["out"]
    return out


# revision 35
# speedup vs baseline: 1.0118x; 1.0051x over previous
"""AdaptiveSpikingAttention on 8 TRN2 NeuronCores (Bass/Tile).

Sharding: the 4096 (batch, seq) rows are split across 8 cores — core c owns
batch c//2, half c%2 (512 rows). Projections, gate MLPs and spike counting
are row-local; the two cores of a batch exchange k/v spike counts with a
pair AllGather before the attention.

Key transform: the 20-step LIF spike recurrence acc(x, T) is a monotone
step function of x whose <=20 jump points depend only on (alpha, beta, T).
The jump points are bisected on the host from the scalar parameters; on
device each element needs 20 compares against per-row thresholds instead
of a sequential 20-step recurrence.

Softmax: scores only ever exist transposed ([col, row]); the row bound
M_i = scale*(q_i . kmean) + C is folded into the score matmul as an extra
contraction row. row-max >= row-mean keeps the denominator well away from
underflow, and C centers the exp arguments in fp32 range.
"""

import sys
import numpy as np

sys.path.insert(0, "/opt/trn_rl_repo")

import concourse.bass as bass
import concourse.bacc as bacc
import concourse.tile as tile
import concourse.mybir as mybir
from concourse.bass_utils import run_bass_kernel_spmd
from concourse.masks import make_identity

f32 = mybir.dt.float32
bf16 = mybir.dt.bfloat16
P = 128
R = 512           # rows per core
E = 512
H, D = 8, 64
S = 1024
NK = 20           # thresholds per tensor
T_MAX = 20
CSHIFT = 114.0    # exp-range centering constant
SCALE = float(D) ** -0.5

_compiled = None


# ----------------------------------------------------------------- host math
def _build_thr_table(alpha, beta):
    """thr[T-1, k-1]: smallest f32 x with count(x, T) >= k (64.0 if never)."""
    alpha = np.float32(alpha)
    beta = np.float32(beta)

    def counts(xs, T):
        xs = xs.astype(np.float32)
        v = np.zeros_like(xs)
        i = np.zeros_like(xs)
        acc = np.zeros_like(xs)
        for t in range(T_MAX):
            a = np.float32(1.0) if t < T else np.float32(0.0)
            i = alpha * i + xs * a
            v = beta * v + i
            s = (v >= 1.0).astype(np.float32)
            v = v * (1.0 - s)
            acc = acc + s * a
        return acc

    thr = np.full((T_MAX, T_MAX), np.float32(64.0), np.float32)
    for T in range(1, T_MAX + 1):
        los = np.full(T, -3, np.float32)
        his = np.full(T, 6, np.float32)
        ks = np.arange(1, T + 1)
        for _ in range(60):
            mids = ((los.astype(np.float64) + his) / 2).astype(np.float32)
            ge = counts(mids, T) >= ks
            his = np.where(ge, mids, his)
            los = np.where(ge, los, mids)
        thr[T - 1, :T] = his
    return thr


# -------------------------------------------------------------- device build
def _build_program():
    nc = bacc.Bacc("TRN2", target_bir_lowering=False, debug=False,
                   enable_asserts=True, num_devices=8)
    A = mybir.AluOpType
    AF = mybir.ActivationFunctionType
    X = mybir.AxisListType.X

    def dram(name, shape, dt=f32, kind="ExternalInput"):
        return nc.dram_tensor(name, shape, dt, kind=kind)

    xT_d = dram("xT", [E, R])
    Wq_d = dram("Wq", [E, E])
    Wk_d = dram("Wk", [E, E])
    Wv_d = dram("Wv", [E, E])
    Wo_d = dram("Wo_s", [E, E], bf16)
    bo_d = dram("bo_row", [1, E])
    gw1_d = dram("gW1", [E, 128]); gb1_d = dram("gb1", [128, 1])
    gg_d = dram("gg", [128, 1]); gbe_d = dram("gbe", [128, 1])
    gw2_d = dram("gW2", [128, 64]); gb2_d = dram("gb2", [64, 1])
    gw3_d = dram("gW3", [64, 1]); gb3_d = dram("gb3", [1, 1])
    cw1_d = dram("cW1", [E, 64]); cb1_d = dram("cb1", [64, 1])
    cg_d = dram("cg", [64, 1]); cbe_d = dram("cbe", [64, 1])
    cw2_d = dram("cW2", [64, 32]); cb2_d = dram("cb2", [32, 1])
    cw3_d = dram("cW3", [32, 1]); cb3_d = dram("cb3", [1, 1])
    pos_d = dram("pos_row", [1, R])
    tbl_d = dram("tbl_all", [NK, 3 * NK])
    tau1_d = dram("tau1", [NK, 1])
    tau2_d = dram("tau2", [NK, 1])
    out_d = dram("out", [R, E], kind="ExternalOutput")

    with tile.TileContext(nc) as tc:
        with (
            tc.tile_pool(name="w", bufs=1) as wpool,
            tc.tile_pool(name="sb", bufs=2) as pool,
            tc.tile_pool(name="row", bufs=1) as rowp,
            tc.tile_pool(name="cnt", bufs=1) as cpool,
            tc.tile_pool(name="psA", bufs=2, space="PSUM") as psA,
            tc.tile_pool(name="psB", bufs=2, space="PSUM") as psB,
            tc.tile_pool(name="psC", bufs=1, space="PSUM") as psC,
            tc.tile_pool(name="dram", bufs=1, space="DRAM") as dpool,
        ):
            # ---------- load everything
            xT = wpool.tile([P, 4, R], f32)
            for c in range(4):
                nc.sync.dma_start(xT[:, c], xT_d[c * P:(c + 1) * P, :])
            Ws = {}
            for qi, (nm, d) in enumerate((("q", Wq_d), ("k", Wk_d),
                                          ("v", Wv_d))):
                W = wpool.tile([P, 4, E], f32, tag=f"W{nm}")
                eng = (nc.gpsimd, nc.scalar, nc.gpsimd)[qi]
                for c in range(4):
                    eng.dma_start(W[:, c], d[c * P:(c + 1) * P, :])
                Ws[nm] = W
            Wo = wpool.tile([D, H, E], bf16)
            for h in range(H):
                nc.scalar.dma_start(Wo[:, h], Wo_d[h * D:(h + 1) * D, :])
            bo_b = wpool.tile([P, E], f32)
            nc.sync.dma_start(bo_b[:], bo_d[0:1, :].to_broadcast((P, E)))

            gw1 = wpool.tile([P, 4, 128], f32)
            for c in range(4):
                nc.sync.dma_start(gw1[:, c], gw1_d[c * P:(c + 1) * P, :])
            cw1 = wpool.tile([P, 4, 64], f32)
            for c in range(4):
                nc.sync.dma_start(cw1[:, c], cw1_d[c * P:(c + 1) * P, :])
            gw2 = wpool.tile([P, 64], f32)
            nc.sync.dma_start(gw2[:], gw2_d[:, :])
            cw2 = wpool.tile([64, 32], f32)
            nc.sync.dma_start(cw2[:], cw2_d[:, :])
            gw3 = wpool.tile([64, 1], f32)
            nc.sync.dma_start(gw3[:], gw3_d[:, :])
            cw3 = wpool.tile([32, 1], f32)
            nc.sync.dma_start(cw3[:], cw3_d[:, :])
            smalls = {}
            for nm, d, pp in (("gb1", gb1_d, 128), ("gg", gg_d, 128),
                              ("gbe", gbe_d, 128), ("gb2", gb2_d, 64),
                              ("gb3", gb3_d, 1), ("cb1", cb1_d, 64),
                              ("cg", cg_d, 64), ("cbe", cbe_d, 64),
                              ("cb2", cb2_d, 32), ("cb3", cb3_d, 1)):
                t = wpool.tile([pp, 1], f32, tag=nm)
                nc.sync.dma_start(t[:], d[:, :])
                smalls[nm] = t
            pos_row = wpool.tile([1, R], f32)
            nc.sync.dma_start(pos_row[:], pos_d[:, :])
            tbl = wpool.tile([NK, 3 * NK], f32)
            nc.sync.dma_start(tbl[:], tbl_d[:, :])
            tau1 = wpool.tile([NK, 1], f32)
            nc.sync.dma_start(tau1[:], tau1_d[:, :])
            tau2 = wpool.tile([NK, 1], f32)
            nc.sync.dma_start(tau2[:], tau2_d[:, :])

            ident = wpool.tile([P, P], bf16)
            make_identity(nc, ident[:])
            ident_f = wpool.tile([P, P], f32)
            make_identity(nc, ident_f[:])
            ones1_20 = wpool.tile([1, NK], f32)
            nc.vector.memset(ones1_20[:], 1.0)
            ones20c = wpool.tile([NK, 1], bf16)
            nc.vector.memset(ones20c[:], 1.0)
            negC = wpool.tile([P, 1], f32)
            nc.vector.memset(negC[:], -CSHIFT)

            # ---------- gate MLP (feature-major layout: [feat, rows])
            def mlp_branch(w1, b1, g, be, w2, b2, w3, b3, f1, f2, tg):
                h1_ps = psA.tile([f1, R], f32, tag="m")
                for c in range(4):
                    nc.tensor.matmul(h1_ps[:], w1[:, c], xT[:, c],
                                     start=(c == 0), stop=(c == 3))
                h1 = pool.tile([f1, R], f32, tag="mh1")
                nc.vector.tensor_scalar(h1[:], h1_ps[:], b1[:], None,
                                        op0=A.add)
                sq = pool.tile([f1, R], f32, tag="msq")
                nc.vector.tensor_tensor(sq[:], h1[:], h1[:], op=A.mult)
                onesf = rowp.tile([f1, 1], f32, tag="mof")
                nc.vector.memset(onesf[:], 1.0)
                mu_ps = psA.tile([1, R], f32, tag="m")
                nc.tensor.matmul(mu_ps[:], onesf[:], h1[:],
                                 start=True, stop=True)
                s2_ps = psA.tile([1, R], f32, tag="m")
                nc.tensor.matmul(s2_ps[:], onesf[:], sq[:],
                                 start=True, stop=True)
                mu = rowp.tile([1, R], f32, tag="mmu")
                nc.vector.tensor_scalar(mu[:], mu_ps[:], 1.0 / f1, None,
                                        op0=A.mult)
                m2 = rowp.tile([1, R], f32, tag="mm2")
                nc.vector.tensor_scalar(m2[:], s2_ps[:], 1.0 / f1, None,
                                        op0=A.mult)
                var = rowp.tile([1, R], f32, tag="mvar")
                nc.vector.tensor_tensor(var[:], mu[:], mu[:], op=A.mult)
                nc.vector.tensor_tensor(var[:], m2[:], var[:], op=A.subtract)
                eps = rowp.tile([1, 1], f32, tag="meps")
                nc.vector.memset(eps[:], 1e-5)
                std = rowp.tile([1, R], f32, tag="mstd")
                nc.scalar.activation(std[:], var[:], AF.Sqrt, bias=eps[:])
                rstd = rowp.tile([1, R], f32, tag="mrstd")
                nc.vector.reciprocal(rstd[:], std[:])
                onesr = rowp.tile([1, f1], f32, tag="mor")
                nc.vector.memset(onesr[:], 1.0)
                mb_ps = psA.tile([f1, R], f32, tag="m")
                nc.tensor.matmul(mb_ps[:], onesr[:], mu[:],
                                 start=True, stop=True)
                rb_ps = psB.tile([f1, R], f32, tag="b", bufs=1)
                nc.tensor.matmul(rb_ps[:], onesr[:], rstd[:],
                                 start=True, stop=True)
                hc = pool.tile([f1, R], f32, tag="mhc")
                nc.vector.tensor_tensor(hc[:], h1[:], mb_ps[:], op=A.subtract)
                nc.vector.tensor_tensor(hc[:], hc[:], rb_ps[:], op=A.mult)
                hn = pool.tile([f1, R], f32, tag="mhn")
                nc.vector.tensor_scalar(hn[:], hc[:], g[:], be[:],
                                        op0=A.mult, op1=A.add)
                nc.vector.tensor_scalar(hn[:], hn[:], 0.0, None, op0=A.max)
                h2_ps = psA.tile([f2, R], f32, tag="m")
                nc.tensor.matmul(h2_ps[:], w2[:], hn[:], start=True, stop=True)
                h2 = pool.tile([f2, R], f32, tag="mh2")
                nc.vector.tensor_scalar(h2[:], h2_ps[:], b2[:], 0.0,
                                        op0=A.add, op1=A.max)
                h3_ps = psB.tile([1, R], f32, tag="b", bufs=1)
                nc.tensor.matmul(h3_ps[:], w3[:], h2[:], start=True, stop=True)
                sig = rowp.tile([1, R], f32, tag=f"{tg}sig")
                nc.scalar.activation(sig[:], h3_ps[:], AF.Sigmoid, bias=b3[:])
                return sig

            g3 = mlp_branch(gw1, smalls["gb1"], smalls["gg"], smalls["gbe"],
                            gw2, smalls["gb2"], gw3, smalls["gb3"],
                            128, 64, "g")
            c3 = mlp_branch(cw1, smalls["cb1"], smalls["cg"], smalls["cbe"],
                            cw2, smalls["cb2"], cw3, smalls["cb3"],
                            64, 32, "c")

            # y = 20 * ((0.7 g + 0.3 c) * pos), mirroring reference rounding
            c3s = rowp.tile([1, R], f32)
            nc.vector.tensor_scalar(c3s[:], c3[:], 0.3, None, op0=A.mult)
            y = rowp.tile([1, R], f32)
            nc.vector.scalar_tensor_tensor(out=y[:], in0=g3[:], scalar=0.7,
                                           in1=c3s[:], op0=A.mult, op1=A.add)
            nc.vector.tensor_tensor(y[:], y[:], pos_row[:], op=A.mult)
            nc.vector.tensor_scalar(y[:], y[:], 20.0, None, op0=A.mult)

            # staircase -> T -> one-hot, all [20, rows]
            yb_ps = psA.tile([NK, R], f32, tag="m")
            nc.tensor.matmul(yb_ps[:], ones1_20[:], y[:], start=True, stop=True)
            yb = rowp.tile([NK, R], f32)
            nc.vector.tensor_copy(yb[:], yb_ps[:])
            St = rowp.tile([NK, R], bf16)
            nc.vector.tensor_scalar(St[:], yb[:], tau1[:], None, op0=A.is_gt)
            T_ps = psB.tile([1, R], f32, tag="b", bufs=1)
            nc.tensor.matmul(T_ps[:], ones20c[:], St[:], start=True, stop=True)
            T_row = rowp.tile([1, R], f32)
            nc.vector.tensor_copy(T_row[:], T_ps[:])
            Tb_ps = psA.tile([NK, R], f32, tag="m")
            nc.tensor.matmul(Tb_ps[:], ones1_20[:], T_row[:],
                             start=True, stop=True)
            Tb = rowp.tile([NK, R], f32)
            nc.vector.tensor_copy(Tb[:], Tb_ps[:])
            Ot = rowp.tile([NK, R], f32)
            nc.vector.tensor_scalar(Ot[:], Tb[:], tau2[:], None, op0=A.is_equal)

            # per row-tile thresholds G [128, 60] (and negated, for ACT Sign)
            G = cpool.tile([P, 4, 3 * NK], f32)
            nG = cpool.tile([P, 4, 3 * NK], f32)
            for rt in range(4):
                g_ps = psB.tile([P, 3 * NK], f32, tag="b", bufs=1)
                nc.tensor.matmul(g_ps[:], Ot[:, rt * P:(rt + 1) * P], tbl[:],
                                 start=True, stop=True)
                nc.vector.tensor_copy(G[:, rt], g_ps[:])
                nc.vector.tensor_scalar(nG[:, rt], g_ps[:], -1.0, None,
                                        op0=A.mult)

            # ---------- QKV projections (fp32) + spike counts
            # k and v first (they feed the pair AllGather), q overlaps it.
            # compares: k on ACT (Sign), q/v on DVE (is_ge);
            # adds: q/v PE identity-accumulate, k DVE bf16 chain.
            cnt = {nm: cpool.tile([P, 4, E], bf16, tag=f"cnt_{nm}",
                                  name=f"cnt_{nm}")
                   for nm in ("q", "k", "v")}
            colbase = {"q": 0, "k": NK, "v": 2 * NK}
            kTl = cpool.tile([P, 4, R], bf16, tag="kTl")
            qA = cpool.tile([D + 1, H, R], bf16, tag="qA")

            def project(nm, rt):
                pj_ps = psA.tile([P, E], f32, tag="m", name="pj_ps")
                for c in range(4):
                    nc.tensor.matmul(pj_ps[:],
                                     xT[:, c, rt * P:(rt + 1) * P],
                                     Ws[nm][:, c],
                                     start=(c == 0), stop=(c == 3))
                t = pool.tile([P, E], f32, tag="pj_k" if nm == "k" else "pj_qv", name=f"pj_{nm}")
                nc.vector.tensor_copy(t[:], pj_ps[:])
                return t

            def counts_pe(nm, rt, pjt):
                """DVE compares + PE identity-accumulate."""
                cb = colbase[nm]
                acc_ps = psC.tile([P, E], f32, tag="acc", name="acc_ps")
                for k in range(NK):
                    ck = pool.tile([P, E], bf16, tag=f"ck{k % 2}", name="ck")
                    nc.vector.tensor_scalar(
                        ck[:], pjt[:], G[:, rt, cb + k:cb + k + 1],
                        None, op0=A.is_ge)
                    nc.tensor.matmul(acc_ps[:], ident[:], ck[:],
                                     start=(k == 0), stop=(k == NK - 1),
                                     skip_group_check=True)
                nc.scalar.copy(cnt[nm][:, rt], acc_ps[:])

            def counts_k(rt, pjt):
                """ACT Sign compares + DVE bf16 chain adds + affine fix."""
                cb = colbase["k"]
                kacc = None
                sks = []
                for k in range(NK):
                    sk = pool.tile([P, E], bf16, tag=f"sk{k % 2}", name="sk")
                    nc.scalar.sign(sk[:], pjt[:],
                                   bias=nG[:, rt, cb + k:cb + k + 1])
                    sks.append(sk)
                    if len(sks) == 2:
                        na = pool.tile([P, E], bf16, tag="ka", name="ka")
                        if kacc is None:
                            nc.vector.tensor_tensor(na[:], sks[0][:],
                                                    sks[1][:], op=A.add)
                        else:
                            nc.vector.tensor_tensor(na[:], kacc[:], sks[0][:],
                                                    op=A.add)
                            na2 = pool.tile([P, E], bf16, tag="kb", name="kb")
                            nc.vector.tensor_tensor(na2[:], na[:], sks[1][:],
                                                    op=A.add)
                            na = na2
                        kacc = na
                        sks = []
                nc.vector.tensor_scalar(cnt["k"][:, rt], kacc[:], 0.5, 10.0,
                                        op0=A.mult, op1=A.add)

            def transpose_k(rt):
                for ec in range(4):
                    t_ps = psB.tile([P, P], bf16, tag="b", bufs=1, name="t_ps")
                    nc.tensor.matmul(
                        t_ps[:], cnt["k"][:, rt, ec * P:(ec + 1) * P],
                        ident[:], is_transpose=True)
                    nc.scalar.copy(kTl[:, ec, rt * P:(rt + 1) * P], t_ps[:])

            def transpose_q(rt):
                for ec in range(4):
                    t_ps = psB.tile([P, P], bf16, tag="b", bufs=1, name="t_ps")
                    nc.tensor.matmul(
                        t_ps[:], cnt["q"][:, rt, ec * P:(ec + 1) * P],
                        ident[:], is_transpose=True)
                    nc.vector.tensor_copy(
                        qA[0:D, 2 * ec, rt * P:(rt + 1) * P], t_ps[0:D, :])
                    nc.vector.tensor_copy(
                        qA[0:D, 2 * ec + 1, rt * P:(rt + 1) * P],
                        t_ps[D:2 * D, :])

            snd_k = dpool.tile([4, P, E], bf16)
            snd_v = dpool.tile([4, P, E], bf16)
            rcv_k = dpool.tile([2, 4, P, E], bf16)
            rcv_v = dpool.tile([2, 4, P, E], bf16)
            kA = cpool.tile([D + 1, H, S], bf16, tag="kA")
            nc.gpsimd.memset(kA[:], 1.0)
            v_aug = cpool.tile([P, 8, H, D + 1], bf16, tag="v_aug")
            nc.gpsimd.memset(v_aug[:], 1.0)
            # k and v counts first (collective inputs); send per-rt
            for rt in range(4):
                pk = project("k", rt)
                pv_ = project("v", rt)
                counts_k(rt, pk)
                counts_pe("v", rt, pv_)
                transpose_k(rt)
                nc.sync.dma_start(
                    snd_k[rt].rearrange("p (ec rc) -> p ec rc", ec=4, rc=P),
                    kTl[:, :, rt * P:(rt + 1) * P])
                nc.scalar.dma_start(snd_v[rt], cnt["v"][:, rt])

            # k gathers first: its result is needed first (ksum, scores)
            nc.gpsimd.collective_compute(
                "AllGather", mybir.AluOpType.bypass,
                ins=[snd_k.opt()], outs=[rcv_k.opt()],
                replica_groups=[[0, 1], [2, 3], [4, 5], [6, 7]],
            )
            for rank in range(2):
                for rt in range(4):
                    nc.sync.dma_start(
                        kA[0:D, :, rank * R + rt * P:
                           rank * R + (rt + 1) * P].rearrange(
                            "d (ec h2) rc -> d ec h2 rc", ec=4, h2=2),
                        rcv_k[rank, rt].rearrange(
                            "(h2 d) (ec rc) -> d ec h2 rc",
                            h2=2, d=D, ec=4, rc=P))
            ksum = cpool.tile([D, H, 1], f32, tag="ksum")
            ksum_bf = cpool.tile([D, H, 1], bf16, tag="ksum_bf")
            for h in range(H):
                nc.vector.reduce_sum(ksum[:, h], kA[0:D, h, :], axis=X)
                nc.vector.tensor_copy(ksum_bf[:, h], ksum[:, h])
            nc.gpsimd.collective_compute(
                "AllGather", mybir.AluOpType.bypass,
                ins=[snd_v.opt()], outs=[rcv_v.opt()],
                replica_groups=[[0, 1], [2, 3], [4, 5], [6, 7]],
            )
            for rank in range(2):
                for j in range(4):
                    eng = (nc.sync, nc.scalar, nc.gpsimd, nc.sync)[j]
                    eng.dma_start(
                        v_aug[:, rank * 4 + j, :, 0:D],
                        rcv_v[rank, j].rearrange(
                            "p (h d) -> p h d", h=H, d=D))

            # q counts overlap the collectives: 10 ACT signs (+-1) and
            # 10 DVE doubled indicators (0/2) PE-accumulated; count=(T+10)/2
            for rt in range(4):
                pq = project("q", rt)
                cb = colbase["q"]
                acc_ps = psC.tile([P, E], f32, tag="acc", name="acc_ps")
                for k in range(NK):
                    ck = pool.tile([P, E], bf16, tag=f"ck{k % 2}", name="ck")
                    if k < 10:
                        nc.scalar.sign(ck[:], pq[:],
                                       bias=nG[:, rt, cb + k:cb + k + 1])
                    else:
                        nc.vector.tensor_scalar(
                            ck[:], pq[:], G[:, rt, cb + k:cb + k + 1],
                            2.0, op0=A.is_ge, op1=A.mult)
                    nc.tensor.matmul(acc_ps[:], ident[:], ck[:],
                                     start=(k == 0), stop=(k == NK - 1),
                                     skip_group_check=True)
                nc.vector.tensor_scalar(cnt["q"][:, rt], acc_ps[:], 0.5, 5.0,
                                        op0=A.mult, op1=A.add)
                transpose_q(rt)

            # ---------- shifted-bound aug row: -(q . ksum)/S per head
            for h in range(H):
                aug_ps = psB.tile([1, R], f32, tag="b", bufs=1)
                nc.tensor.matmul(aug_ps[:], ksum_bf[:, h], qA[0:D, h, :],
                                 start=True, stop=True)
                nc.scalar.activation(qA[D:D + 1, h, :], aug_ps[:],
                                     AF.Copy, scale=-1.0 / S)

            # ---------- attention: scores^T -> exp -> transposed PV
            # pv_T [65, rows] = v_aug.T @ w^T per head; row 64 is the softmax
            # denominator (from v_aug's ones column).
            UT = cpool.tile([D, H, R], bf16, tag="UT")
            den_hold = cpool.tile([D + 1, H, R], bf16, tag="den_hold")
            for h in range(H):
                w_h = []
                for pb in range(4):
                    sc_ps = psA.tile([P, 2, R], f32, tag="m", name="sc_ps")
                    for half in range(2):
                        cb_ = pb * 2 + half
                        nc.tensor.matmul(sc_ps[:, half],
                                         kA[:, h, cb_ * P:(cb_ + 1) * P],
                                         qA[:, h, :], start=True, stop=True,
                                         skip_group_check=True)
                    w_sb = pool.tile([P, 2, R], bf16, tag=f"w{pb}_{h % 2}", bufs=1,
                                     name="w_sb")
                    nc.scalar.activation(w_sb[:], sc_ps[:], AF.Exp,
                                         scale=SCALE, bias=negC[:])
                    w_h.append(w_sb)
                pvt_ps = psB.tile([D + 1, R], f32, tag="pv", bufs=2)
                for cc in range(8):
                    nc.tensor.matmul(pvt_ps[:], v_aug[:, cc, h],
                                     w_h[cc // 2][:, cc % 2],
                                     start=(cc == 0), stop=(cc == 7),
                                     skip_group_check=True)
                nc.vector.tensor_copy(UT[:, h, :], pvt_ps[0:D, :])
                nc.vector.tensor_copy(den_hold[D:D + 1, h, :],
                                      pvt_ps[D:D + 1, :])

            # transpose denominators to row-major, reciprocal, back to rows
            recT = cpool.tile([P, 4, H], f32, tag="recT")
            for rt in range(4):
                dT = pool.tile([P, H], bf16, tag="dT", name="dT")
                for h in range(H):
                    m_ps = psB.tile([P, 1], bf16, tag="b", bufs=1, name="m_ps")
                    nc.tensor.matmul(
                        m_ps[:],
                        den_hold[D:D + 1, h, rt * P:(rt + 1) * P],
                        ident[D:D + 1, D:D + 1], is_transpose=True)
                    nc.vector.tensor_copy(dT[:, h:h + 1], m_ps[:])
                nc.vector.reciprocal(recT[:, rt, :], dT[:])
            rrow = cpool.tile([H, R], bf16, tag="rrow")
            for rt in range(4):
                r_ps = psB.tile([H, P], bf16, tag="b", bufs=1, name="r_ps")
                rT16 = pool.tile([P, H], bf16, tag="rT16", name="rT16")
                nc.vector.tensor_copy(rT16[:], recT[:, rt, :])
                nc.tensor.matmul(r_ps[:], rT16[:], ident[:],
                                 is_transpose=True)
                nc.vector.tensor_copy(rrow[:, rt * P:(rt + 1) * P], r_ps[:])
            rrow_d = dpool.tile([H, R], bf16)
            nc.sync.dma_start(rrow_d[:], rrow[:])
            for h in range(H):
                rb = pool.tile([D, R], bf16, tag=f"rb{h % 2}", bufs=1,
                               name="rb")
                nc.sync.dma_start(rb[:],
                                  rrow_d[h:h + 1, :].to_broadcast((D, R)))
                nc.vector.tensor_tensor(UT[:, h, :], UT[:, h, :], rb[:],
                                        op=A.mult)

            # out = sum_h UT_h.T @ Wo[h-rows] + bo   (K=64 per head)
            for rt in range(4):
                o_ps = psA.tile([P, E], f32, tag="m", name="o_ps")
                for h in range(H):
                    nc.tensor.matmul(o_ps[:],
                                     UT[:, h, rt * P:(rt + 1) * P],
                                     Wo[:, h, :],
                                     start=(h == 0), stop=(h == H - 1))
                o_sb = pool.tile([P, E], f32, tag="o_sb", name="o_sb")
                nc.vector.tensor_tensor(o_sb[:], o_ps[:], bo_b[:], op=A.add)
                nc.sync.dma_start(out_d[rt * P:(rt + 1) * P, :], o_sb[:])

    nc.compile()
    return nc


# ------------------------------------------------------------------- driver
def kernel(**inputs) -> np.ndarray:
    import ml_dtypes
    global _compiled
    inp = {k: np.asarray(v) for k, v in inputs.items()}
    x = inp["x"].astype(np.float32)
    B = x.shape[0]

    thr_q = _build_thr_table(inp["alpha_q"], inp["beta_q"])
    thr_k = _build_thr_table(inp["alpha_k"], inp["beta_k"])
    thr_v = _build_thr_table(inp["alpha_v"], inp["beta_v"])
    tbl_all = np.concatenate([thr_q, thr_k, thr_v], axis=1)  # [20, 60]

    pos_full = np.linspace(0.8, 1.2, S, dtype=np.float32)
    tau1 = np.array([-1.0] + [float(j) for j in range(1, NK)],
                    np.float32).reshape(NK, 1)
    tau2 = np.arange(1, NK + 1, dtype=np.float32).reshape(NK, 1)
    Wo_s16 = (inp["Wo"].astype(np.float64) / T_MAX).astype(
        np.float32).astype(ml_dtypes.bfloat16)

    def col(a):
        return np.ascontiguousarray(np.asarray(a, np.float32).reshape(-1, 1))

    common = {
        "Wq": np.ascontiguousarray(inp["Wq"].astype(np.float32)),
        "Wk": np.ascontiguousarray(inp["Wk"].astype(np.float32)),
        "Wv": np.ascontiguousarray(inp["Wv"].astype(np.float32)),
        "Wo_s": np.ascontiguousarray(Wo_s16),
        "bo_row": np.ascontiguousarray(
            inp["bo"].astype(np.float32).reshape(1, E)),
        "gW1": np.ascontiguousarray(inp["gW1"].astype(np.float32)),
        "gb1": col(inp["gb1"]), "gg": col(inp["gg"]), "gbe": col(inp["gbe"]),
        "gW2": np.ascontiguousarray(inp["gW2"].astype(np.float32)),
        "gb2": col(inp["gb2"]),
        "gW3": np.ascontiguousarray(inp["gW3"].astype(np.float32)),
        "gb3": col(inp["gb3"]),
        "cW1": np.ascontiguousarray(inp["cW1"].astype(np.float32)),
        "cb1": col(inp["cb1"]), "cg": col(inp["cg"]), "cbe": col(inp["cbe"]),
        "cW2": np.ascontiguousarray(inp["cW2"].astype(np.float32)),
        "cb2": col(inp["cb2"]),
        "cW3": np.ascontiguousarray(inp["cW3"].astype(np.float32)),
        "cb3": col(inp["cb3"]),
        "tbl_all": np.ascontiguousarray(tbl_all),
        "tau1": tau1, "tau2": tau2,
    }

    in_maps = []
    for c in range(8):
        b, half = c // 2, c % 2
        rows = slice(half * R, half * R + R)
        m = dict(common)
        m["xT"] = np.ascontiguousarray(x[b, rows].T)
        m["pos_row"] = np.ascontiguousarray(pos_full[rows].reshape(1, R))
        in_maps.append(m)

    if _compiled is None:
        _compiled = _build_program()
    nc = _compiled

    res = run_bass_kernel_spmd(nc, in_maps, core_ids=list(range(8)))

    out = np.zeros((B, S, E), np.float32)
    for c in range(8):
        b, half = c // 2, c % 2
        out[b, half * R:(half + 1) * R, :] = res.results[c]["out"]
    return out
